# revision 24
# baseline (speedup 1.0000x reference)
"""AdaConv (nn_AdaConv_46445776339355) — 8-core TRN2 Bass kernel.

Strategy
--------
Data-parallel over batch N=8: core n owns sample n end-to-end for the heavy
instance-norm + grouped-conv work.  The kernel *generator* (dw_w is 256 MiB)
is tensor-parallel: core j holds the output-channel shard j of dw_w / pwk_w,
computes the generated kernels for ALL samples on its shard, and an AllToAll
routes each sample's kernels to its owning core.

Algebraic fusions (all computed on device):
  * pointwise o depthwise = one fused per-group kernel  F_t = P @ W_t
  * instance norm folded into the fused kernels:
        y = sum_t F_t @ ((x-mu)/sigma)_pad = sum_t (F_t/sigma_ci) @ x_pad - B
    with B = sum_t (F_t/sigma) @ mu  (position independent, reflect-pad safe)
  * biases (dw_b, pwk_b, pwb_b) folded in via K=1 matmul rows.

The grouped conv (8 groups of 64->64 ch, 3x3) runs as 4 concurrent 64x64
matmuls in the 4 PE-array quadrants (tile_position packing) => full 128x128
PE utilization, bf16, 9 shifted-AP taps accumulating in PSUM.
"""

import sys
import numpy as np

sys.path.insert(0, "/opt/trn_rl_repo")

import ml_dtypes

BF16 = ml_dtypes.bfloat16

# ---------------- problem constants (hardcoded per the harness contract) ----
N = 8            # batch == number of cores
C = 512          # channels
H = W = 128
HW = H * W       # 16384
PW = W + 2       # 130 padded
PA = PW * PW     # 16900
SD = 512         # style dim
NG = 8           # groups
GS = 64          # group size (channels per group)
KDW = SD * 4     # 2048 contraction dim of the dw generator
OSH = 4096       # dw/pwk output-channel shard per core (32768/8); == one group block
NTAP = 9
EPS = 1e-5
VAR_CORR = float(HW) / float(HW - 1)  # ddof=1 correction

# device output channel order: per pair of groups (2h, 2h+1) natural, the odd
# pairs (pB) have their two 64-blocks swapped (quadrant output packing).
TAU_BLOCK = [0, 1, 3, 2, 4, 5, 7, 6]  # true 64-block of device 64-block d


def _host_prep(style_encoding, predicted, dw_w, dw_b, pwk_w, pwk_b, pwb_w, pwb_b):
    """Pure data-movement / dtype-cast host prep. Returns per-core input maps."""
    f32 = np.float32
    se = np.asarray(style_encoding, f32)
    pred = np.asarray(predicted, f32)

    # --- patches for the dw generator conv: reflect pad 1, 2x2 windows s=2 ---
    sep = np.pad(se, ((0, 0), (0, 0), (1, 1), (1, 1)), mode="reflect")  # (8,512,6,6)
    blocks = sep.reshape(N, SD, 3, 2, 3, 2)  # [n,c,oy,ky,ox,kx]
    patches = np.ascontiguousarray(
        blocks.transpose(1, 3, 5, 0, 2, 4).reshape(KDW, N * NTAP)
    ).astype(BF16)  # [(c,ky,kx), (n,oy,ox)] = [2048, 72]

    # --- dw generator weights, transposed + sharded on output channels ---
    dwt_full = np.ascontiguousarray(dw_w.reshape(C * GS, KDW).T).astype(BF16)  # [2048, 32768]
    dwb_full = np.asarray(dw_b, f32).reshape(1, C * GS).astype(BF16)

    # --- pwk: permute columns to (g, cm, co2) so the gathered row IS P^T ---
    pwk_t = np.asarray(pwk_w, f32).reshape(NG, GS, GS, SD)  # [g, co2, cm, sd]
    pwkt_full = np.ascontiguousarray(
        pwk_t.transpose(3, 0, 2, 1).reshape(SD, C * GS)
    ).astype(BF16)  # [sd, (g, cm, co2)]
    pwkb_full = (
        np.asarray(pwk_b, f32).reshape(NG, GS, GS).transpose(0, 2, 1).reshape(1, C * GS)
    ).astype(BF16)

    # --- pwb: transposed, columns in DEVICE channel order tau ---
    tau_rows = np.concatenate([np.arange(GS) + t * GS for t in TAU_BLOCK])  # [512]
    pwbt = np.ascontiguousarray(np.asarray(pwb_w, f32)[tau_rows, :].T).astype(BF16)  # [sd, out_dev]
    pwbb = np.asarray(pwb_b, f32)[tau_rows].reshape(1, C).astype(BF16)

    # --- styleT for sd computation on device: [c, (n, px)] f32 ---
    styleT = np.ascontiguousarray(se.transpose(1, 0, 2, 3).reshape(SD, N * 16)).astype(f32)

    in_maps = []
    for j in range(N):
        pp = np.pad(pred[j], ((0, 0), (1, 1), (1, 1)), mode="reflect").reshape(C, PA)
        sel = np.zeros((128, N), f32)
        sel[:, j] = 1.0
        in_maps.append(
            dict(
                pred_pad=np.ascontiguousarray(pp).astype(BF16),
                patches=patches,
                styleT=styleT,
                sel=sel,
                dwt=np.ascontiguousarray(dwt_full[:, j * OSH:(j + 1) * OSH]),
                dwb=np.ascontiguousarray(dwb_full[:, j * OSH:(j + 1) * OSH]),
                pwkt=np.ascontiguousarray(pwkt_full[:, j * OSH:(j + 1) * OSH]),
                pwkb=np.ascontiguousarray(pwkb_full[:, j * OSH:(j + 1) * OSH]),
                pwbt=pwbt,
                pwbb=pwbb,
            )
        )
    return in_maps


def _unshard(results):
    """results[j]['out'] is [512, 16384] f32 in device channel order."""
    out = np.empty((N, C, H, W), np.float32)
    for j in range(N):
        dev = np.asarray(results[j]["out"], np.float32).reshape(C, H, W)
        for d, t in enumerate(TAU_BLOCK):
            out[j, t * GS:(t + 1) * GS] = dev[d * GS:(d + 1) * GS]
    return out


# how many 512-px N-tiles are accumulated per psum set before evacuation
CONV_NT = 2


def build_nc():
    from concourse import bacc, mybir, tile
    from contextlib import ExitStack

    dt = mybir.dt
    AF = mybir.ActivationFunctionType
    ALU = mybir.AluOpType

    nc = bacc.Bacc(num_devices=N)

    pred_pad = nc.declare_dram_parameter("pred_pad", [C, PA], dt.bfloat16, isOutput=False)
    patches = nc.declare_dram_parameter("patches", [KDW, N * NTAP], dt.bfloat16, isOutput=False)
    styleT = nc.declare_dram_parameter("styleT", [SD, N * 16], dt.float32, isOutput=False)
    sel = nc.declare_dram_parameter("sel", [128, N], dt.float32, isOutput=False)
    dwt = nc.declare_dram_parameter("dwt", [KDW, OSH], dt.bfloat16, isOutput=False)
    dwb = nc.declare_dram_parameter("dwb", [1, OSH], dt.bfloat16, isOutput=False)
    pwkt = nc.declare_dram_parameter("pwkt", [SD, OSH], dt.bfloat16, isOutput=False)
    pwkb = nc.declare_dram_parameter("pwkb", [1, OSH], dt.bfloat16, isOutput=False)
    pwbt = nc.declare_dram_parameter("pwbt", [SD, C], dt.bfloat16, isOutput=False)
    pwbb = nc.declare_dram_parameter("pwbb", [1, C], dt.bfloat16, isOutput=False)
    out_dev = nc.declare_dram_parameter("out", [C, HW], dt.float32, isOutput=True)

    replica = [list(range(N))]

    with tile.TileContext(nc) as tc, ExitStack() as ctx:
        dram = ctx.enter_context(tc.tile_pool(name="dram", bufs=1, space="DRAM"))
        gen_dw = dram.tile([N, NTAP, OSH], dt.float32, tag="gdw")
        gen_dw_out = dram.tile([N, NTAP, OSH], dt.float32, tag="gdwo")
        gen_pk = dram.tile([N, OSH], dt.float32, tag="gpk")
        gen_pk_out = dram.tile([N, OSH], dt.float32, tag="gpko")

        const_p = ctx.enter_context(tc.tile_pool(name="const", bufs=1))
        pred_p = ctx.enter_context(tc.tile_pool(name="pred", bufs=1))
        stats_p = ctx.enter_context(tc.tile_pool(name="stats", bufs=2))
        fker_p = ctx.enter_context(tc.tile_pool(name="fker", bufs=1))
        fload_p = ctx.enter_context(tc.tile_pool(name="fload", bufs=3))
        stage_p = ctx.enter_context(tc.tile_pool(name="stage", bufs=4))

        # ------------------------------------------------ constant-ish loads
        pt = const_p.tile([128, 16, N * NTAP], dt.bfloat16, tag="pt")
        nc.sync.dma_start(out=pt[:], in_=patches.rearrange("(kt p) m -> p kt m", p=128))

        st = const_p.tile([128, 4, N * 16], dt.float32, tag="st")
        nc.sync.dma_start(out=st[:], in_=styleT.rearrange("(kt p) m -> p kt m", p=128))

        sel_sb = const_p.tile([128, N], dt.float32, tag="sel")
        nc.sync.dma_start(out=sel_sb[:], in_=sel[:, :])

        pwbt_sb = const_p.tile([128, 4, C], dt.bfloat16, tag="pwbt")
        nc.sync.dma_start(out=pwbt_sb[:], in_=pwbt.rearrange("(kt p) m -> p kt m", p=128))

        pwbb_sb = const_p.tile([1, C], dt.bfloat16, tag="pwbb")
        nc.sync.dma_start(out=pwbb_sb[:], in_=pwbb[:, :])

        ones = const_p.tile([1, 128], dt.bfloat16, tag="ones")
        nc.vector.memset(ones[:], 1.0)

        # ------------------------------------------------ predicted (padded, bf16)
        # pair 0 loads first (stats p0 starts early); pairs 1-3 are issued
        # later so the dwt weight stream isn't starved at kernel start.
        img = [pred_p.tile([128, PA], dt.bfloat16, name=f"img{p}", tag=f"img{p}")
               for p in range(4)]
        img_loaded = [False] * 4

        def load_img(p):
            if not img_loaded[p]:
                nc.gpsimd.dma_start(out=img[p][:], in_=pred_pad[p * 128:(p + 1) * 128, :])
                img_loaded[p] = True

        load_img(0)

        # ------------------------------------------------ sd = mean(style, px)
        sdf = const_p.tile([128, 4, N], dt.float32, tag="sdf")
        sdb = const_p.tile([128, 4, N], dt.bfloat16, tag="sdb")
        sdnb = const_p.tile([128, 4], dt.bfloat16, tag="sdnb")  # own-sample column
        tmp8 = stats_p.tile([128, N], dt.float32, tag="tmp8")
        sdn_f = const_p.tile([128, 4], dt.float32, tag="sdnf")
        for kt in range(4):
            nc.vector.tensor_reduce(
                out=sdf[:, kt, :],
                in_=st[:, kt, :].rearrange("p (n x) -> p n x", x=16),
                axis=mybir.AxisListType.X,
                op=ALU.add,
            )
            nc.vector.tensor_scalar(
                out=sdb[:, kt, :], in0=sdf[:, kt, :], scalar1=1.0 / 16.0,
                scalar2=None, op0=ALU.mult,
            )
            # own sample's sd column (via one-hot sel): sdn = sum_n sdf[:,n]*sel[:,n]
            nc.vector.tensor_tensor(
                out=tmp8[:], in0=sdf[:, kt, :], in1=sel_sb[:], op=ALU.mult
            )
            nc.vector.tensor_reduce(
                out=sdn_f[:, kt:kt + 1], in_=tmp8[:], axis=mybir.AxisListType.X, op=ALU.add
            )
        nc.vector.tensor_scalar(
            out=sdnb[:], in0=sdn_f[:], scalar1=1.0 / 16.0, scalar2=None, op0=ALU.mult
        )

        # ------------------------------------------------ generator phase (PE)
        with tc.tile_pool(name="psgen", bufs=8, space="PSUM") as psum_g, \
             tc.tile_pool(name="wstream", bufs=2) as wstream_p, \
             tc.tile_pool(name="gstg", bufs=2) as gstg_p:
            # dw generator
            ps_dw = [psum_g.tile([128, 512], dt.float32, name=f"dwg{b}", tag="g")
                     for b in range(8)]
            dma_engs = [nc.sync, nc.scalar]
            for kt in range(16):
                wt = wstream_p.tile([128, OSH], dt.bfloat16, name="wt", tag="w")
                dma_engs[kt % 2].dma_start(out=wt[:], in_=dwt[kt * 128:(kt + 1) * 128, :])
                for b in range(8):
                    nc.tensor.matmul(
                        out=ps_dw[b][:N * NTAP, :],
                        lhsT=pt[:, kt, :],
                        rhs=wt[:, b * 512:(b + 1) * 512],
                        start=(kt == 0), stop=False,
                    )
            for b in range(8):
                bt = gstg_p.tile([1, 512], dt.bfloat16, name="bt", tag="bias")
                nc.sync.dma_start(out=bt[:], in_=dwb[0:1, b * 512:(b + 1) * 512])
                nc.tensor.matmul(
                    out=ps_dw[b][:N * NTAP, :],
                    lhsT=ones[:1, :N * NTAP],
                    rhs=bt[:1, :],
                    start=False, stop=True,
                )
                gsb = gstg_p.tile([N * NTAP, 512], dt.float32, name="gsb", tag="gs")
                nc.scalar.copy(out=gsb[:], in_=ps_dw[b][:N * NTAP, :])
                nc.sync.dma_start(
                    out=gen_dw[:, :, b * 512:(b + 1) * 512],
                    in_=gsb[:, :],
                )

            # AllToAll for the dw kernels fires early: overlaps pwk/pwb gen
            nc.gpsimd.collective_compute(
                "AllToAll",
                ALU.bypass,
                replica_groups=replica,
                ins=[gen_dw[:, :, :].opt()],
                outs=[gen_dw_out[:, :, :].opt()],
            )

            # pwk generator
            ps_pk = [psum_g.tile([128, 512], dt.float32, name=f"pkg{b}", tag="g")
                     for b in range(8)]
            for kt in range(4):
                wt = wstream_p.tile([128, OSH], dt.bfloat16, name="wt", tag="w")
                dma_engs[kt % 2].dma_start(out=wt[:], in_=pwkt[kt * 128:(kt + 1) * 128, :])
                for b in range(8):
                    nc.tensor.matmul(
                        out=ps_pk[b][:N, :],
                        lhsT=sdb[:, kt, :],
                        rhs=wt[:, b * 512:(b + 1) * 512],
                        start=(kt == 0), stop=False,
                    )
            for b in range(8):
                bt = gstg_p.tile([1, 512], dt.bfloat16, name="bt", tag="bias")
                nc.sync.dma_start(out=bt[:], in_=pwkb[0:1, b * 512:(b + 1) * 512])
                nc.tensor.matmul(
                    out=ps_pk[b][:N, :],
                    lhsT=ones[:1, :N],
                    rhs=bt[:1, :],
                    start=False, stop=True,
                )
                g2sb = gstg_p.tile([N, 512], dt.float32, name="g2sb", tag="gs")
                nc.scalar.copy(out=g2sb[:], in_=ps_pk[b][:N, :])
                nc.sync.dma_start(
                    out=gen_pk[:, b * 512:(b + 1) * 512], in_=g2sb[:]
                )

            # pwb bias chain (device channel order)
            pwb_sb = const_p.tile([128, 4], dt.float32, tag="pwbv")
            for m in range(4):
                pm = psum_g.tile([128, 512], dt.float32, name="pwbps", tag="g")
                for kt in range(4):
                    nc.tensor.matmul(
                        out=pm[:, 0:1],
                        lhsT=pwbt_sb[:, kt, m * 128:(m + 1) * 128],
                        rhs=sdnb[:, kt:kt + 1],
                        start=(kt == 0), stop=False,
                    )
                nc.tensor.matmul(
                    out=pm[:, 0:1],
                    lhsT=pwbb_sb[:1, m * 128:(m + 1) * 128],
                    rhs=ones[:1, 0:1],
                    start=False, stop=True,
                )
                nc.scalar.copy(out=pwb_sb[:, m:m + 1], in_=pm[:, 0:1])

            nc.gpsimd.collective_compute(
                "AllToAll",
                ALU.bypass,
                replica_groups=replica,
                ins=[gen_pk[:, :].opt()],
                outs=[gen_pk_out[:, :].opt()],
            )

        # ------------------------------------------------ instance-norm stats
        # sum-of-squares on VectorE (STT + accum, 8 chunks); sums on ScalarE
        # (Copy activation + accum_out, 8 chunks); var = E[x^2] - mu^2.
        rstd_sb = const_p.tile([128, 4], dt.float32, tag="rstd")
        muneg_sb = const_p.tile([128, 4], dt.bfloat16, tag="muneg")

        def stats_pair(p):
            view = img[p].rearrange("p (r c) -> p r c", c=PW)
            sum8 = stats_p.tile([128, 8], dt.float32, name="sum8", tag="sum8")
            sqd2 = stats_p.tile([128, 16, 128], dt.float32, name="sqd2", tag="sqd2", bufs=1)
            acc8 = stats_p.tile([128, 8], dt.float32, name="acc8", tag="acc8")
            sqd = stats_p.tile([128, 16, 128], dt.float32, name="sqd", tag="sqd", bufs=1)
            for j in range(8):
                xs = view[:, 1 + 16 * j:1 + 16 * (j + 1), 1:129]
                nc.vector.scalar_tensor_tensor(
                    out=sqd[:], in0=xs, scalar=1.0, in1=xs,
                    op0=ALU.mult, op1=ALU.mult,
                    accum_out=acc8[:, j:j + 1],
                )
                nc.scalar.activation(
                    out=sqd2[:], in_=xs,
                    func=AF.Copy, bias=0.0, scale=1.0,
                    accum_out=sum8[:, j:j + 1],
                )
            ssum = stats_p.tile([128, 1], dt.float32, name="ssum", tag="ssum")
            nc.vector.tensor_reduce(
                out=ssum[:], in_=sum8[:], axis=mybir.AxisListType.X, op=ALU.add
            )
            ssq = stats_p.tile([128, 1], dt.float32, name="ssq", tag="ssq")
            nc.vector.tensor_reduce(
                out=ssq[:], in_=acc8[:], axis=mybir.AxisListType.X, op=ALU.add
            )
            mu = stats_p.tile([128, 1], dt.float32, name="mu", tag="mu")
            nc.vector.tensor_scalar(
                out=mu[:], in0=ssum[:], scalar1=1.0 / HW, scalar2=None, op0=ALU.mult
            )
            nc.vector.tensor_scalar(
                out=muneg_sb[:, p:p + 1], in0=mu[:], scalar1=-1.0,
                scalar2=None, op0=ALU.mult,
            )
            ex2 = stats_p.tile([128, 1], dt.float32, name="ex2", tag="ex2")
            nc.vector.tensor_scalar(
                out=ex2[:], in0=ssq[:], scalar1=1.0 / HW, scalar2=None, op0=ALU.mult
            )
            mu2 = stats_p.tile([128, 1], dt.float32, name="mu2", tag="mu2")
            nc.vector.tensor_tensor(out=mu2[:], in0=mu[:], in1=mu[:], op=ALU.mult)
            varp = stats_p.tile([128, 1], dt.float32, name="varp", tag="varp")
            nc.vector.tensor_tensor(out=varp[:], in0=ex2[:], in1=mu2[:], op=ALU.subtract)
            vtmp = stats_p.tile([128, 1], dt.float32, name="vtmp", tag="sm2")
            nc.vector.tensor_scalar(
                out=vtmp[:], in0=varp[:], scalar1=VAR_CORR, scalar2=EPS,
                op0=ALU.mult, op1=ALU.add,
            )
            stdt = stats_p.tile([128, 1], dt.float32, name="stdt", tag="sm3")
            nc.scalar.sqrt(stdt[:], vtmp[:])
            nc.vector.reciprocal(out=rstd_sb[:, p:p + 1], in_=stdt[:])

        # ------------------------------------------------ per-half: F build + conv
        with tc.tile_pool(name="pssml", bufs=2, space="PSUM") as psum_s, \
             tc.tile_pool(name="psconv", bufs=6, space="PSUM") as psum_c:
            for h in range(2):
                pA, pB = 2 * h, 2 * h + 1
                load_img(pA)
                load_img(pB)
                stats_pair(pA)
                stats_pair(pB)

                # fused, scaled kernels F~ : [part(ci), tap, geo, co2] bf16
                fsb = fker_p.tile([128, NTAP, 2, GS], dt.bfloat16,
                                  name=f"fsb{h}", tag=f"fsb{h}")
                for t in range(NTAP):
                    w128 = fload_p.tile([128, 2, GS], dt.float32, name="wld", tag="wld")
                    p128 = fload_p.tile([128, 2, GS], dt.float32, name="pld", tag="pld")
                    nc.sync.dma_start(
                        out=w128[0:64, :, :],
                        in_=gen_dw_out[4 * h:4 * h + 3:2, t, :].rearrange(
                            "g (cm ci) -> cm g ci", ci=GS),
                    )
                    nc.sync.dma_start(
                        out=w128[64:128, :, :],
                        in_=gen_dw_out[4 * h + 1:4 * h + 4:2, t, :].rearrange(
                            "g (cm ci) -> cm g ci", ci=GS),
                    )
                    nc.sync.dma_start(
                        out=p128[0:64, :, :],
                        in_=gen_pk_out[4 * h:4 * h + 3:2, :].rearrange(
                            "g (cm co) -> cm g co", co=GS),
                    )
                    nc.sync.dma_start(
                        out=p128[64:128, :, :],
                        in_=gen_pk_out[4 * h + 1:4 * h + 4:2, :].rearrange(
                            "g (cm co) -> cm g co", co=GS),
                    )
                    psA = psum_s.tile([128, 512], dt.float32, name="fbA", tag="s")
                    psB = psum_s.tile([128, 512], dt.float32, name="fbB", tag="s")
                    # F^T = W^T @ P^T  (fp32), per group
                    nc.tensor.matmul(out=psA[0:64, :GS], lhsT=w128[0:64, 0, :],
                                     rhs=p128[0:64, 0, :], start=True, stop=True)
                    nc.tensor.matmul(out=psA[64:128, :GS], lhsT=w128[64:128, 0, :],
                                     rhs=p128[64:128, 0, :], start=True, stop=True)
                    nc.tensor.matmul(out=psB[0:64, :GS], lhsT=w128[0:64, 1, :],
                                     rhs=p128[0:64, 1, :], start=True, stop=True)
                    nc.tensor.matmul(out=psB[64:128, :GS], lhsT=w128[64:128, 1, :],
                                     rhs=p128[64:128, 1, :], start=True, stop=True)
                    # scale by rstd (per input channel) + cast bf16
                    nc.vector.tensor_scalar(out=fsb[0:64, t, 0, :], in0=psA[0:64, :GS],
                                            scalar1=rstd_sb[0:64, pA:pA + 1],
                                            scalar2=None, op0=ALU.mult)
                    nc.vector.tensor_scalar(out=fsb[64:128, t, 0, :], in0=psA[64:128, :GS],
                                            scalar1=rstd_sb[64:128, pA:pA + 1],
                                            scalar2=None, op0=ALU.mult)
                    nc.vector.tensor_scalar(out=fsb[0:64, t, 1, :], in0=psB[0:64, :GS],
                                            scalar1=rstd_sb[0:64, pB:pB + 1],
                                            scalar2=None, op0=ALU.mult)
                    nc.vector.tensor_scalar(out=fsb[64:128, t, 1, :], in0=psB[64:128, :GS],
                                            scalar1=rstd_sb[64:128, pB:pB + 1],
                                            scalar2=None, op0=ALU.mult)

                # B bias accumulation (psum holds -B); pair B quadrant-swapped
                bpsA = psum_s.tile([128, 512], dt.float32, name="bpsA", tag="s")
                bpsB = psum_s.tile([128, 512], dt.float32, name="bpsB", tag="s")
                for t in range(NTAP):
                    st_ = (t == 0)
                    sp_ = (t == NTAP - 1)
                    nc.tensor.matmul(out=bpsA[0:64, 0:1], lhsT=fsb[0:64, t, 0, :],
                                     rhs=muneg_sb[0:64, pA:pA + 1], start=st_, stop=sp_,
                                     skip_group_check=True)
                    nc.tensor.matmul(out=bpsA[64:128, 0:1], lhsT=fsb[64:128, t, 0, :],
                                     rhs=muneg_sb[64:128, pA:pA + 1], start=st_, stop=sp_,
                                     skip_group_check=True)
                    nc.tensor.matmul(out=bpsB[64:128, 0:1], lhsT=fsb[0:64, t, 1, :],
                                     rhs=muneg_sb[0:64, pB:pB + 1], start=st_, stop=sp_,
                                     skip_group_check=True)
                    nc.tensor.matmul(out=bpsB[0:64, 0:1], lhsT=fsb[64:128, t, 1, :],
                                     rhs=muneg_sb[64:128, pB:pB + 1], start=st_, stop=sp_,
                                     skip_group_check=True)
                bias_A = const_p.tile([128, 1], dt.float32, name=f"biasA{h}", tag=f"bA{h}")
                bias_B = const_p.tile([128, 1], dt.float32, name=f"biasB{h}", tag=f"bB{h}")
                nc.vector.tensor_tensor(out=bias_A[:], in0=bpsA[:, 0:1],
                                        in1=pwb_sb[:, pA:pA + 1], op=ALU.add)
                nc.vector.tensor_tensor(out=bias_B[:], in0=bpsB[:, 0:1],
                                        in1=pwb_sb[:, pB:pB + 1], op=ALU.add)

                if h == 0:
                    load_img(2)
                    load_img(3)

                # ------------------------------------------------ the conv
                imA = img[pA].rearrange("p (r c) -> p r c", c=PW)
                imB = img[pB].rearrange("p (r c) -> p r c", c=PW)
                NBLK = HW // (512 * CONV_NT)
                for blk in range(NBLK):
                    y0 = blk * 4 * CONV_NT
                    psA_ = [psum_c.tile([128, 512], dt.float32, name="cvA", tag="c")
                            for _ in range(CONV_NT)]
                    psB_ = [psum_c.tile([128, 512], dt.float32, name="cvB", tag="c")
                            for _ in range(CONV_NT)]
                    for t in range(NTAP):
                        ky, kx = t // 3, t % 3
                        st_ = (t == 0)
                        sp_ = (t == NTAP - 1)
                        for nt in range(CONV_NT):
                            y = y0 + 4 * nt
                            rA = imA[:, y + ky:y + ky + 4, kx:kx + 128]
                            rB = imB[:, y + ky:y + ky + 4, kx:kx + 128]
                            nc.tensor.matmul(out=psA_[nt][0:64, :], lhsT=fsb[0:64, t, 0, :],
                                             rhs=rA[0:64], start=st_, stop=sp_,
                                             skip_group_check=True)
                            nc.tensor.matmul(out=psA_[nt][64:128, :], lhsT=fsb[64:128, t, 0, :],
                                             rhs=rA[64:128], start=st_, stop=sp_,
                                             skip_group_check=True)
                            nc.tensor.matmul(out=psB_[nt][64:128, :], lhsT=fsb[0:64, t, 1, :],
                                             rhs=rB[0:64], start=st_, stop=sp_,
                                             skip_group_check=True)
                            nc.tensor.matmul(out=psB_[nt][0:64, :], lhsT=fsb[64:128, t, 1, :],
                                             rhs=rB[64:128], start=st_, stop=sp_,
                                             skip_group_check=True)
                    # evacuate + bias; ScalarE takes pair A, VectorE pair B
                    stA = stage_p.tile([128, 512 * CONV_NT], dt.float32, name="stA", tag="stg")
                    stB = stage_p.tile([128, 512 * CONV_NT], dt.float32, name="stB", tag="stg")
                    for nt in range(CONV_NT):
                        nc.scalar.activation(
                            out=stA[:, nt * 512:(nt + 1) * 512], in_=psA_[nt][:],
                            func=AF.Identity, bias=bias_A[:, 0:1], scale=1.0,
                        )
                        nc.scalar.activation(
                            out=stB[:, nt * 512:(nt + 1) * 512], in_=psB_[nt][:],
                            func=AF.Identity, bias=bias_B[:, 0:1], scale=1.0,
                        )
                    px0 = y0 * 128
                    nc.sync.dma_start(
                        out=out_dev[pA * 128:(pA + 1) * 128, px0:px0 + 512 * CONV_NT],
                        in_=stA[:],
                    )
                    nc.sync.dma_start(
                        out=out_dev[pB * 128:(pB + 1) * 128, px0:px0 + 512 * CONV_NT],
                        in_=stB[:],
                    )

    nc.compile()
    return nc


_NC_CACHE = {}


def kernel(**inputs) -> np.ndarray:
    from concourse.bass_utils import run_bass_kernel_spmd

    in_maps = _host_prep(**inputs)
    if "nc" not in _NC_CACHE:
        _NC_CACHE["nc"] = build_nc()
    nc = _NC_CACHE["nc"]
    res = run_bass_kernel_spmd(nc, in_maps, core_ids=list(range(N)))
    return _unshard(res.results)


if __name__ == "__main__":
    import jax

    import reference

    with jax.default_device(jax.devices("cpu")[0]):
        inputs = {k: np.asarray(v) for k, v in reference.setup_inputs().items()}
        expected = np.asarray(reference.reference(**inputs))
    actual = kernel(**inputs)
    err = np.sqrt(((actual - expected) ** 2).mean()) / np.sqrt((expected ** 2).mean())
    print("Relative error:", err)


# revision 26
# speedup vs baseline: 1.1031x; 1.1031x over previous
"""AdaConv (nn_AdaConv_46445776339355) — 8-core TRN2 Bass kernel.

Strategy
--------
Data-parallel over batch N=8: core n owns sample n end-to-end for the heavy
instance-norm + grouped-conv work.  The kernel *generator* (dw_w is 256 MiB)
is tensor-parallel: core j holds the output-channel shard j of dw_w / pwk_w,
computes the generated kernels for ALL samples on its shard, and an AllToAll
routes each sample's kernels to its owning core.

Algebraic fusions (all computed on device):
  * pointwise o depthwise = one fused per-group kernel  F_t = P @ W_t
  * instance norm folded into the fused kernels:
        y = sum_t F_t @ ((x-mu)/sigma)_pad = sum_t (F_t/sigma_ci) @ x_pad - B
    with B = sum_t (F_t/sigma) @ mu  (position independent, reflect-pad safe)
  * biases (dw_b, pwk_b, pwb_b) folded in via K=1 matmul rows.

The grouped conv (8 groups of 64->64 ch, 3x3) runs as 4 concurrent 64x64
matmuls in the 4 PE-array quadrants (tile_position packing) => full 128x128
PE utilization, bf16, 9 shifted-AP taps accumulating in PSUM.
"""

import sys
import numpy as np

sys.path.insert(0, "/opt/trn_rl_repo")

import ml_dtypes

BF16 = ml_dtypes.bfloat16

# ---------------- problem constants (hardcoded per the harness contract) ----
N = 8            # batch == number of cores
C = 512          # channels
H = W = 128
HW = H * W       # 16384
PW = W + 2       # 130 padded
PA = PW * PW     # 16900
SD = 512         # style dim
NG = 8           # groups
GS = 64          # group size (channels per group)
KDW = SD * 4     # 2048 contraction dim of the dw generator
OSH = 4096       # dw/pwk output-channel shard per core (32768/8); == one group block
NTAP = 9
EPS = 1e-5
VAR_CORR = float(HW) / float(HW - 1)  # ddof=1 correction

# device output channel order: per pair of groups (2h, 2h+1) natural, the odd
# pairs (pB) have their two 64-blocks swapped (quadrant output packing).
TAU_BLOCK = [0, 1, 3, 2, 4, 5, 7, 6]  # true 64-block of device 64-block d


def _host_prep(style_encoding, predicted, dw_w, dw_b, pwk_w, pwk_b, pwb_w, pwb_b):
    """Pure data-movement / dtype-cast host prep. Returns per-core input maps."""
    f32 = np.float32
    se = np.asarray(style_encoding, f32)
    pred = np.asarray(predicted, f32)

    # --- patches for the dw generator conv: reflect pad 1, 2x2 windows s=2 ---
    sep = np.pad(se, ((0, 0), (0, 0), (1, 1), (1, 1)), mode="reflect")  # (8,512,6,6)
    blocks = sep.reshape(N, SD, 3, 2, 3, 2)  # [n,c,oy,ky,ox,kx]
    patches = np.ascontiguousarray(
        blocks.transpose(1, 3, 5, 0, 2, 4).reshape(KDW, N * NTAP)
    ).astype(BF16)  # [(c,ky,kx), (n,oy,ox)] = [2048, 72]

    # --- dw generator weights, transposed + sharded on output channels ---
    dwt_full = np.ascontiguousarray(dw_w.reshape(C * GS, KDW).T).astype(BF16)  # [2048, 32768]
    dwb_full = np.asarray(dw_b, f32).reshape(1, C * GS).astype(BF16)

    # --- pwk: permute columns to (g, cm, co2) so the gathered row IS P^T ---
    pwk_t = np.asarray(pwk_w, f32).reshape(NG, GS, GS, SD)  # [g, co2, cm, sd]
    pwkt_full = np.ascontiguousarray(
        pwk_t.transpose(3, 0, 2, 1).reshape(SD, C * GS)
    ).astype(BF16)  # [sd, (g, cm, co2)]
    pwkb_full = (
        np.asarray(pwk_b, f32).reshape(NG, GS, GS).transpose(0, 2, 1).reshape(1, C * GS)
    ).astype(BF16)

    # --- pwb: transposed, columns in DEVICE channel order tau ---
    tau_rows = np.concatenate([np.arange(GS) + t * GS for t in TAU_BLOCK])  # [512]
    pwbt = np.ascontiguousarray(np.asarray(pwb_w, f32)[tau_rows, :].T).astype(BF16)  # [sd, out_dev]
    pwbb = np.asarray(pwb_b, f32)[tau_rows].reshape(1, C).astype(BF16)

    # --- styleT for sd computation on device: [c, (n, px)] f32 ---
    styleT = np.ascontiguousarray(se.transpose(1, 0, 2, 3).reshape(SD, N * 16)).astype(f32)

    in_maps = []
    for j in range(N):
        pp = np.pad(pred[j], ((0, 0), (1, 1), (1, 1)), mode="reflect").reshape(C, PA)
        sel = np.zeros((128, N), f32)
        sel[:, j] = 1.0
        in_maps.append(
            dict(
                pred_pad=np.ascontiguousarray(pp).astype(BF16),
                patches=patches,
                styleT=styleT,
                sel=sel,
                dwt=np.ascontiguousarray(dwt_full[:, j * OSH:(j + 1) * OSH]),
                dwb=np.ascontiguousarray(dwb_full[:, j * OSH:(j + 1) * OSH]),
                pwkt=np.ascontiguousarray(pwkt_full[:, j * OSH:(j + 1) * OSH]),
                pwkb=np.ascontiguousarray(pwkb_full[:, j * OSH:(j + 1) * OSH]),
                pwbt=pwbt,
                pwbb=pwbb,
            )
        )
    return in_maps


def _unshard(results):
    """results[j]['out'] is [512, 16384] f32 in device channel order."""
    out = np.empty((N, C, H, W), np.float32)
    for j in range(N):
        dev = np.asarray(results[j]["out"], np.float32).reshape(C, H, W)
        for d, t in enumerate(TAU_BLOCK):
            out[j, t * GS:(t + 1) * GS] = dev[d * GS:(d + 1) * GS]
    return out


# how many 512-px N-tiles are accumulated per psum set before evacuation
CONV_NT = 2


def build_nc():
    from concourse import bacc, mybir, tile
    from contextlib import ExitStack

    dt = mybir.dt
    AF = mybir.ActivationFunctionType
    ALU = mybir.AluOpType

    nc = bacc.Bacc(num_devices=N)

    pred_pad = nc.declare_dram_parameter("pred_pad", [C, PA], dt.bfloat16, isOutput=False)
    patches = nc.declare_dram_parameter("patches", [KDW, N * NTAP], dt.bfloat16, isOutput=False)
    styleT = nc.declare_dram_parameter("styleT", [SD, N * 16], dt.float32, isOutput=False)
    sel = nc.declare_dram_parameter("sel", [128, N], dt.float32, isOutput=False)
    dwt = nc.declare_dram_parameter("dwt", [KDW, OSH], dt.bfloat16, isOutput=False)
    dwb = nc.declare_dram_parameter("dwb", [1, OSH], dt.bfloat16, isOutput=False)
    pwkt = nc.declare_dram_parameter("pwkt", [SD, OSH], dt.bfloat16, isOutput=False)
    pwkb = nc.declare_dram_parameter("pwkb", [1, OSH], dt.bfloat16, isOutput=False)
    pwbt = nc.declare_dram_parameter("pwbt", [SD, C], dt.bfloat16, isOutput=False)
    pwbb = nc.declare_dram_parameter("pwbb", [1, C], dt.bfloat16, isOutput=False)
    out_dev = nc.declare_dram_parameter("out", [C, HW], dt.float32, isOutput=True)

    replica = [list(range(N))]

    with tile.TileContext(nc) as tc, ExitStack() as ctx:
        dram = ctx.enter_context(tc.tile_pool(name="dram", bufs=1, space="DRAM"))
        gen_all = dram.tile([N, 10, OSH], dt.bfloat16, tag="gall")
        gen_out = dram.tile([N, 10, OSH], dt.bfloat16, tag="gout")

        const_p = ctx.enter_context(tc.tile_pool(name="const", bufs=1))
        pred_p = ctx.enter_context(tc.tile_pool(name="pred", bufs=1))
        stats_p = ctx.enter_context(tc.tile_pool(name="stats", bufs=2))
        fker_p = ctx.enter_context(tc.tile_pool(name="fker", bufs=1))
        fload_p = ctx.enter_context(tc.tile_pool(name="fload", bufs=3))
        stage_p = ctx.enter_context(tc.tile_pool(name="stage", bufs=4))

        # ------------------------------------------------ constant-ish loads
        pt = const_p.tile([128, 16, N * NTAP], dt.bfloat16, tag="pt")
        nc.sync.dma_start(out=pt[:], in_=patches.rearrange("(kt p) m -> p kt m", p=128))

        st = const_p.tile([128, 4, N * 16], dt.float32, tag="st")
        nc.sync.dma_start(out=st[:], in_=styleT.rearrange("(kt p) m -> p kt m", p=128))

        sel_sb = const_p.tile([128, N], dt.float32, tag="sel")
        nc.sync.dma_start(out=sel_sb[:], in_=sel[:, :])

        pwbt_sb = const_p.tile([128, 4, C], dt.bfloat16, tag="pwbt")
        nc.sync.dma_start(out=pwbt_sb[:], in_=pwbt.rearrange("(kt p) m -> p kt m", p=128))

        pwbb_sb = const_p.tile([1, C], dt.bfloat16, tag="pwbb")
        nc.sync.dma_start(out=pwbb_sb[:], in_=pwbb[:, :])

        ones = const_p.tile([1, 128], dt.bfloat16, tag="ones")
        nc.vector.memset(ones[:], 1.0)

        # ------------------------------------------------ predicted (padded, bf16)
        # pair 0 loads first (stats p0 starts early); pairs 1-3 are issued
        # later so the dwt weight stream isn't starved at kernel start.
        img = [pred_p.tile([128, PA], dt.bfloat16, name=f"img{p}", tag=f"img{p}")
               for p in range(4)]
        img_loaded = [False] * 4

        def load_img(p):
            if not img_loaded[p]:
                nc.gpsimd.dma_start(out=img[p][:], in_=pred_pad[p * 128:(p + 1) * 128, :])
                img_loaded[p] = True

        load_img(0)

        # ------------------------------------------------ sd = mean(style, px)
        sdf = const_p.tile([128, 4, N], dt.float32, tag="sdf")
        sdb = const_p.tile([128, 4, N], dt.bfloat16, tag="sdb")
        sdnb = const_p.tile([128, 4], dt.bfloat16, tag="sdnb")  # own-sample column
        tmp8 = stats_p.tile([128, N], dt.float32, tag="tmp8")
        sdn_f = const_p.tile([128, 4], dt.float32, tag="sdnf")
        for kt in range(4):
            nc.vector.tensor_reduce(
                out=sdf[:, kt, :],
                in_=st[:, kt, :].rearrange("p (n x) -> p n x", x=16),
                axis=mybir.AxisListType.X,
                op=ALU.add,
            )
            nc.vector.tensor_scalar(
                out=sdb[:, kt, :], in0=sdf[:, kt, :], scalar1=1.0 / 16.0,
                scalar2=None, op0=ALU.mult,
            )
            # own sample's sd column (via one-hot sel): sdn = sum_n sdf[:,n]*sel[:,n]
            nc.vector.tensor_tensor(
                out=tmp8[:], in0=sdf[:, kt, :], in1=sel_sb[:], op=ALU.mult
            )
            nc.vector.tensor_reduce(
                out=sdn_f[:, kt:kt + 1], in_=tmp8[:], axis=mybir.AxisListType.X, op=ALU.add
            )
        nc.vector.tensor_scalar(
            out=sdnb[:], in0=sdn_f[:], scalar1=1.0 / 16.0, scalar2=None, op0=ALU.mult
        )

        # ------------------------------------------------ generator phase (PE)
        with tc.tile_pool(name="psgen", bufs=8, space="PSUM") as psum_g, \
             tc.tile_pool(name="wstream", bufs=2) as wstream_p, \
             tc.tile_pool(name="gstg", bufs=2) as gstg_p:
            # dw generator
            ps_dw = [psum_g.tile([128, 512], dt.float32, name=f"dwg{b}", tag="g")
                     for b in range(8)]
            for kt in range(16):
                wt = wstream_p.tile([128, OSH], dt.bfloat16, name="wt", tag="w")
                nc.sync.dma_start(out=wt[:], in_=dwt[kt * 128:(kt + 1) * 128, :])
                for b in range(8):
                    nc.tensor.matmul(
                        out=ps_dw[b][:N * NTAP, :],
                        lhsT=pt[:, kt, :],
                        rhs=wt[:, b * 512:(b + 1) * 512],
                        start=(kt == 0), stop=False,
                    )
            for b in range(8):
                bt = gstg_p.tile([1, 512], dt.bfloat16, name="bt", tag="bias")
                nc.sync.dma_start(out=bt[:], in_=dwb[0:1, b * 512:(b + 1) * 512])
                nc.tensor.matmul(
                    out=ps_dw[b][:N * NTAP, :],
                    lhsT=ones[:1, :N * NTAP],
                    rhs=bt[:1, :],
                    start=False, stop=True,
                )
                gsb = gstg_p.tile([N * NTAP, 512], dt.bfloat16, name="gsb", tag="gs")
                nc.scalar.copy(out=gsb[:], in_=ps_dw[b][:N * NTAP, :])
                nc.sync.dma_start(
                    out=gen_all[:, 0:NTAP, b * 512:(b + 1) * 512],
                    in_=gsb[:, :],
                )

            # pwk generator
            ps_pk = [psum_g.tile([128, 512], dt.float32, name=f"pkg{b}", tag="g")
                     for b in range(8)]
            for kt in range(4):
                wt = wstream_p.tile([128, OSH], dt.bfloat16, name="wt", tag="w")
                nc.sync.dma_start(out=wt[:], in_=pwkt[kt * 128:(kt + 1) * 128, :])
                for b in range(8):
                    nc.tensor.matmul(
                        out=ps_pk[b][:N, :],
                        lhsT=sdb[:, kt, :],
                        rhs=wt[:, b * 512:(b + 1) * 512],
                        start=(kt == 0), stop=False,
                    )
            for b in range(8):
                bt = gstg_p.tile([1, 512], dt.bfloat16, name="bt", tag="bias")
                nc.sync.dma_start(out=bt[:], in_=pwkb[0:1, b * 512:(b + 1) * 512])
                nc.tensor.matmul(
                    out=ps_pk[b][:N, :],
                    lhsT=ones[:1, :N],
                    rhs=bt[:1, :],
                    start=False, stop=True,
                )
                g2sb = gstg_p.tile([N, 512], dt.bfloat16, name="g2sb", tag="gs")
                nc.scalar.copy(out=g2sb[:], in_=ps_pk[b][:N, :])
                nc.sync.dma_start(
                    out=gen_all[:, NTAP, b * 512:(b + 1) * 512], in_=g2sb[:]
                )

            # pwb bias chain (device channel order)
            pwb_sb = const_p.tile([128, 4], dt.float32, tag="pwbv")
            for m in range(4):
                pm = psum_g.tile([128, 512], dt.float32, name="pwbps", tag="g")
                for kt in range(4):
                    nc.tensor.matmul(
                        out=pm[:, 0:1],
                        lhsT=pwbt_sb[:, kt, m * 128:(m + 1) * 128],
                        rhs=sdnb[:, kt:kt + 1],
                        start=(kt == 0), stop=False,
                    )
                nc.tensor.matmul(
                    out=pm[:, 0:1],
                    lhsT=pwbb_sb[:1, m * 128:(m + 1) * 128],
                    rhs=ones[:1, 0:1],
                    start=False, stop=True,
                )
                nc.scalar.copy(out=pwb_sb[:, m:m + 1], in_=pm[:, 0:1])

            nc.gpsimd.collective_compute(
                "AllToAll",
                ALU.bypass,
                replica_groups=replica,
                ins=[gen_all[:, :, :].opt()],
                outs=[gen_out[:, :, :].opt()],
            )

        # ------------------------------------------------ instance-norm stats
        # sum-of-squares on VectorE (STT + accum, 8 chunks); sums on ScalarE
        # (Copy activation + accum_out, 8 chunks); var = E[x^2] - mu^2.
        rstd_sb = const_p.tile([128, 4], dt.float32, tag="rstd")
        muneg_sb = const_p.tile([128, 4], dt.bfloat16, tag="muneg")

        def stats_pair(p):
            view = img[p].rearrange("p (r c) -> p r c", c=PW)
            sum8 = stats_p.tile([128, 8], dt.float32, name="sum8", tag="sum8")
            sqd2 = stats_p.tile([128, 16, 128], dt.float32, name="sqd2", tag="sqd2", bufs=1)
            acc8 = stats_p.tile([128, 8], dt.float32, name="acc8", tag="acc8")
            sqd = stats_p.tile([128, 16, 128], dt.float32, name="sqd", tag="sqd", bufs=1)
            for j in range(8):
                xs = view[:, 1 + 16 * j:1 + 16 * (j + 1), 1:129]
                nc.vector.scalar_tensor_tensor(
                    out=sqd[:], in0=xs, scalar=1.0, in1=xs,
                    op0=ALU.mult, op1=ALU.mult,
                    accum_out=acc8[:, j:j + 1],
                )
                nc.scalar.activation(
                    out=sqd2[:], in_=xs,
                    func=AF.Copy, bias=0.0, scale=1.0,
                    accum_out=sum8[:, j:j + 1],
                )
            ssum = stats_p.tile([128, 1], dt.float32, name="ssum", tag="ssum")
            nc.vector.tensor_reduce(
                out=ssum[:], in_=sum8[:], axis=mybir.AxisListType.X, op=ALU.add
            )
            ssq = stats_p.tile([128, 1], dt.float32, name="ssq", tag="ssq")
            nc.vector.tensor_reduce(
                out=ssq[:], in_=acc8[:], axis=mybir.AxisListType.X, op=ALU.add
            )
            mu = stats_p.tile([128, 1], dt.float32, name="mu", tag="mu")
            nc.vector.tensor_scalar(
                out=mu[:], in0=ssum[:], scalar1=1.0 / HW, scalar2=None, op0=ALU.mult
            )
            nc.vector.tensor_scalar(
                out=muneg_sb[:, p:p + 1], in0=mu[:], scalar1=-1.0,
                scalar2=None, op0=ALU.mult,
            )
            ex2 = stats_p.tile([128, 1], dt.float32, name="ex2", tag="ex2")
            nc.vector.tensor_scalar(
                out=ex2[:], in0=ssq[:], scalar1=1.0 / HW, scalar2=None, op0=ALU.mult
            )
            mu2 = stats_p.tile([128, 1], dt.float32, name="mu2", tag="mu2")
            nc.vector.tensor_tensor(out=mu2[:], in0=mu[:], in1=mu[:], op=ALU.mult)
            varp = stats_p.tile([128, 1], dt.float32, name="varp", tag="varp")
            nc.vector.tensor_tensor(out=varp[:], in0=ex2[:], in1=mu2[:], op=ALU.subtract)
            vtmp = stats_p.tile([128, 1], dt.float32, name="vtmp", tag="sm2")
            nc.vector.tensor_scalar(
                out=vtmp[:], in0=varp[:], scalar1=VAR_CORR, scalar2=EPS,
                op0=ALU.mult, op1=ALU.add,
            )
            stdt = stats_p.tile([128, 1], dt.float32, name="stdt", tag="sm3")
            nc.scalar.sqrt(stdt[:], vtmp[:])
            nc.vector.reciprocal(out=rstd_sb[:, p:p + 1], in_=stdt[:])

        # ------------------------------------------------ per-half: F build + conv
        with tc.tile_pool(name="pssml", bufs=2, space="PSUM") as psum_s, \
             tc.tile_pool(name="psconv", bufs=6, space="PSUM") as psum_c:
            for h in range(2):
                pA, pB = 2 * h, 2 * h + 1
                load_img(pA)
                load_img(pB)
                stats_pair(pA)
                stats_pair(pB)

                # fused, scaled kernels F~ : [part(ci), tap, geo, co2] bf16
                fsb = fker_p.tile([128, NTAP, 2, GS], dt.bfloat16,
                                  name=f"fsb{h}", tag=f"fsb{h}")
                for t in range(NTAP):
                    w128 = fload_p.tile([128, 2, GS], dt.bfloat16, name="wld", tag="wld")
                    p128 = fload_p.tile([128, 2, GS], dt.bfloat16, name="pld", tag="pld")
                    nc.sync.dma_start(
                        out=w128[0:64, :, :],
                        in_=gen_out[4 * h:4 * h + 3:2, t, :].rearrange(
                            "g (cm ci) -> cm g ci", ci=GS),
                    )
                    nc.sync.dma_start(
                        out=w128[64:128, :, :],
                        in_=gen_out[4 * h + 1:4 * h + 4:2, t, :].rearrange(
                            "g (cm ci) -> cm g ci", ci=GS),
                    )
                    nc.sync.dma_start(
                        out=p128[0:64, :, :],
                        in_=gen_out[4 * h:4 * h + 3:2, NTAP, :].rearrange(
                            "g (cm co) -> cm g co", co=GS),
                    )
                    nc.sync.dma_start(
                        out=p128[64:128, :, :],
                        in_=gen_out[4 * h + 1:4 * h + 4:2, NTAP, :].rearrange(
                            "g (cm co) -> cm g co", co=GS),
                    )
                    psA = psum_s.tile([128, 512], dt.float32, name="fbA", tag="s")
                    psB = psum_s.tile([128, 512], dt.float32, name="fbB", tag="s")
                    # F^T = W^T @ P^T  (fp32), per group
                    nc.tensor.matmul(out=psA[0:64, :GS], lhsT=w128[0:64, 0, :],
                                     rhs=p128[0:64, 0, :], start=True, stop=True)
                    nc.tensor.matmul(out=psA[64:128, :GS], lhsT=w128[64:128, 0, :],
                                     rhs=p128[64:128, 0, :], start=True, stop=True)
                    nc.tensor.matmul(out=psB[0:64, :GS], lhsT=w128[0:64, 1, :],
                                     rhs=p128[0:64, 1, :], start=True, stop=True)
                    nc.tensor.matmul(out=psB[64:128, :GS], lhsT=w128[64:128, 1, :],
                                     rhs=p128[64:128, 1, :], start=True, stop=True)
                    # scale by rstd (per input channel) + cast bf16
                    nc.vector.tensor_scalar(out=fsb[0:64, t, 0, :], in0=psA[0:64, :GS],
                                            scalar1=rstd_sb[0:64, pA:pA + 1],
                                            scalar2=None, op0=ALU.mult)
                    nc.vector.tensor_scalar(out=fsb[64:128, t, 0, :], in0=psA[64:128, :GS],
                                            scalar1=rstd_sb[64:128, pA:pA + 1],
                                            scalar2=None, op0=ALU.mult)
                    nc.vector.tensor_scalar(out=fsb[0:64, t, 1, :], in0=psB[0:64, :GS],
                                            scalar1=rstd_sb[0:64, pB:pB + 1],
                                            scalar2=None, op0=ALU.mult)
                    nc.vector.tensor_scalar(out=fsb[64:128, t, 1, :], in0=psB[64:128, :GS],
                                            scalar1=rstd_sb[64:128, pB:pB + 1],
                                            scalar2=None, op0=ALU.mult)

                # B bias accumulation (psum holds -B); pair B quadrant-swapped
                bpsA = psum_s.tile([128, 512], dt.float32, name="bpsA", tag="s")
                bpsB = psum_s.tile([128, 512], dt.float32, name="bpsB", tag="s")
                for t in range(NTAP):
                    st_ = (t == 0)
                    sp_ = (t == NTAP - 1)
                    nc.tensor.matmul(out=bpsA[0:64, 0:1], lhsT=fsb[0:64, t, 0, :],
                                     rhs=muneg_sb[0:64, pA:pA + 1], start=st_, stop=sp_,
                                     skip_group_check=True)
                    nc.tensor.matmul(out=bpsA[64:128, 0:1], lhsT=fsb[64:128, t, 0, :],
                                     rhs=muneg_sb[64:128, pA:pA + 1], start=st_, stop=sp_,
                                     skip_group_check=True)
                    nc.tensor.matmul(out=bpsB[64:128, 0:1], lhsT=fsb[0:64, t, 1, :],
                                     rhs=muneg_sb[0:64, pB:pB + 1], start=st_, stop=sp_,
                                     skip_group_check=True)
                    nc.tensor.matmul(out=bpsB[0:64, 0:1], lhsT=fsb[64:128, t, 1, :],
                                     rhs=muneg_sb[64:128, pB:pB + 1], start=st_, stop=sp_,
                                     skip_group_check=True)
                bias_A = const_p.tile([128, 1], dt.float32, name=f"biasA{h}", tag=f"bA{h}")
                bias_B = const_p.tile([128, 1], dt.float32, name=f"biasB{h}", tag=f"bB{h}")
                nc.vector.tensor_tensor(out=bias_A[:], in0=bpsA[:, 0:1],
                                        in1=pwb_sb[:, pA:pA + 1], op=ALU.add)
                nc.vector.tensor_tensor(out=bias_B[:], in0=bpsB[:, 0:1],
                                        in1=pwb_sb[:, pB:pB + 1], op=ALU.add)

                if h == 0:
                    load_img(2)
                    load_img(3)

                # ------------------------------------------------ the conv
                imA = img[pA].rearrange("p (r c) -> p r c", c=PW)
                imB = img[pB].rearrange("p (r c) -> p r c", c=PW)
                NBLK = HW // (512 * CONV_NT)
                for blk in range(NBLK):
                    y0 = blk * 4 * CONV_NT
                    psA_ = [psum_c.tile([128, 512], dt.float32, name="cvA", tag="c")
                            for _ in range(CONV_NT)]
                    psB_ = [psum_c.tile([128, 512], dt.float32, name="cvB", tag="c")
                            for _ in range(CONV_NT)]
                    for t in range(NTAP):
                        ky, kx = t // 3, t % 3
                        st_ = (t == 0)
                        sp_ = (t == NTAP - 1)
                        for nt in range(CONV_NT):
                            y = y0 + 4 * nt
                            rA = imA[:, y + ky:y + ky + 4, kx:kx + 128]
                            rB = imB[:, y + ky:y + ky + 4, kx:kx + 128]
                            nc.tensor.matmul(out=psA_[nt][0:64, :], lhsT=fsb[0:64, t, 0, :],
                                             rhs=rA[0:64], start=st_, stop=sp_,
                                             skip_group_check=True)
                            nc.tensor.matmul(out=psA_[nt][64:128, :], lhsT=fsb[64:128, t, 0, :],
                                             rhs=rA[64:128], start=st_, stop=sp_,
                                             skip_group_check=True)
                            nc.tensor.matmul(out=psB_[nt][64:128, :], lhsT=fsb[0:64, t, 1, :],
                                             rhs=rB[0:64], start=st_, stop=sp_,
                                             skip_group_check=True)
                            nc.tensor.matmul(out=psB_[nt][0:64, :], lhsT=fsb[64:128, t, 1, :],
                                             rhs=rB[64:128], start=st_, stop=sp_,
                                             skip_group_check=True)
                    # evacuate + bias; ScalarE takes pair A, VectorE pair B
                    stA = stage_p.tile([128, 512 * CONV_NT], dt.float32, name="stA", tag="stg")
                    stB = stage_p.tile([128, 512 * CONV_NT], dt.float32, name="stB", tag="stg")
                    for nt in range(CONV_NT):
                        nc.scalar.activation(
                            out=stA[:, nt * 512:(nt + 1) * 512], in_=psA_[nt][:],
                            func=AF.Identity, bias=bias_A[:, 0:1], scale=1.0,
                        )
                        nc.scalar.activation(
                            out=stB[:, nt * 512:(nt + 1) * 512], in_=psB_[nt][:],
                            func=AF.Identity, bias=bias_B[:, 0:1], scale=1.0,
                        )
                    px0 = y0 * 128
                    nc.sync.dma_start(
                        out=out_dev[pA * 128:(pA + 1) * 128, px0:px0 + 512 * CONV_NT],
                        in_=stA[:],
                    )
                    nc.sync.dma_start(
                        out=out_dev[pB * 128:(pB + 1) * 128, px0:px0 + 512 * CONV_NT],
                        in_=stB[:],
                    )

    nc.compile()
    return nc


_NC_CACHE = {}


def kernel(**inputs) -> np.ndarray:
    from concourse.bass_utils import run_bass_kernel_spmd

    in_maps = _host_prep(**inputs)
    if "nc" not in _NC_CACHE:
        _NC_CACHE["nc"] = build_nc()
    nc = _NC_CACHE["nc"]
    res = run_bass_kernel_spmd(nc, in_maps, core_ids=list(range(N)))
    return _unshard(res.results)


if __name__ == "__main__":
    import jax

    import reference

    with jax.default_device(jax.devices("cpu")[0]):
        inputs = {k: np.asarray(v) for k, v in reference.setup_inputs().items()}
        expected = np.asarray(reference.reference(**inputs))
    actual = kernel(**inputs)
    err = np.sqrt(((actual - expected) ** 2).mean()) / np.sqrt((expected ** 2).mean())
    print("Relative error:", err)


# revision 27
# speedup vs baseline: 1.1319x; 1.0261x over previous
"""AdaConv (nn_AdaConv_46445776339355) — 8-core TRN2 Bass kernel.

Strategy
--------
Data-parallel over batch N=8: core n owns sample n end-to-end for the heavy
instance-norm + grouped-conv work.  The kernel *generator* (dw_w is 256 MiB)
is tensor-parallel: core j holds the output-channel shard j of dw_w / pwk_w,
computes the generated kernels for ALL samples on its shard, and an AllToAll
routes each sample's kernels to its owning core.

Algebraic fusions (all computed on device):
  * pointwise o depthwise = one fused per-group kernel  F_t = P @ W_t
  * instance norm folded into the fused kernels:
        y = sum_t F_t @ ((x-mu)/sigma)_pad = sum_t (F_t/sigma_ci) @ x_pad - B
    with B = sum_t (F_t/sigma) @ mu  (position independent, reflect-pad safe)
  * biases (dw_b, pwk_b, pwb_b) folded in via K=1 matmul rows.

The grouped conv (8 groups of 64->64 ch, 3x3) runs as 4 concurrent 64x64
matmuls in the 4 PE-array quadrants (tile_position packing) => full 128x128
PE utilization, bf16, 9 shifted-AP taps accumulating in PSUM.
"""

import sys
import numpy as np

sys.path.insert(0, "/opt/trn_rl_repo")

import ml_dtypes

BF16 = ml_dtypes.bfloat16

# ---------------- problem constants (hardcoded per the harness contract) ----
N = 8            # batch == number of cores
C = 512          # channels
H = W = 128
HW = H * W       # 16384
PW = W + 2       # 130 padded
PA = PW * PW     # 16900
SD = 512         # style dim
NG = 8           # groups
GS = 64          # group size (channels per group)
KDW = SD * 4     # 2048 contraction dim of the dw generator
OSH = 4096       # dw/pwk output-channel shard per core (32768/8); == one group block
NTAP = 9
EPS = 1e-5
VAR_CORR = float(HW) / float(HW - 1)  # ddof=1 correction

# device output channel order: per pair of groups (2h, 2h+1) natural, the odd
# pairs (pB) have their two 64-blocks swapped (quadrant output packing).
TAU_BLOCK = [0, 1, 3, 2, 4, 5, 7, 6]  # true 64-block of device 64-block d


def _host_prep(style_encoding, predicted, dw_w, dw_b, pwk_w, pwk_b, pwb_w, pwb_b):
    """Pure data-movement / dtype-cast host prep. Returns per-core input maps."""
    f32 = np.float32
    se = np.asarray(style_encoding, f32)
    pred = np.asarray(predicted, f32)

    # --- patches for the dw generator conv: reflect pad 1, 2x2 windows s=2 ---
    sep = np.pad(se, ((0, 0), (0, 0), (1, 1), (1, 1)), mode="reflect")  # (8,512,6,6)
    blocks = sep.reshape(N, SD, 3, 2, 3, 2)  # [n,c,oy,ky,ox,kx]
    patches = np.ascontiguousarray(
        blocks.transpose(1, 3, 5, 0, 2, 4).reshape(KDW, N * NTAP)
    ).astype(BF16)  # [(c,ky,kx), (n,oy,ox)] = [2048, 72]
    # pre-tiled for a contiguous SBUF DMA: [128, kt, m]
    patches_t = np.ascontiguousarray(
        patches.reshape(16, 128, N * NTAP).transpose(1, 0, 2).reshape(128, 16 * N * NTAP)
    )

    # --- dw generator weights, transposed + sharded on output channels ---
    dwt_full = np.ascontiguousarray(dw_w.reshape(C * GS, KDW).T).astype(BF16)  # [2048, 32768]
    dwb_full = np.asarray(dw_b, f32).reshape(1, C * GS).astype(BF16)

    # --- pwk: permute columns to (g, cm, co2) so the gathered row IS P^T ---
    pwk_t = np.asarray(pwk_w, f32).reshape(NG, GS, GS, SD)  # [g, co2, cm, sd]
    pwkt_full = np.ascontiguousarray(
        pwk_t.transpose(3, 0, 2, 1).reshape(SD, C * GS)
    ).astype(BF16)  # [sd, (g, cm, co2)]
    pwkb_full = (
        np.asarray(pwk_b, f32).reshape(NG, GS, GS).transpose(0, 2, 1).reshape(1, C * GS)
    ).astype(BF16)

    # --- pwb: transposed, columns in DEVICE channel order tau ---
    tau_rows = np.concatenate([np.arange(GS) + t * GS for t in TAU_BLOCK])  # [512]
    pwbt = np.ascontiguousarray(np.asarray(pwb_w, f32)[tau_rows, :].T).astype(BF16)  # [sd, out_dev]
    pwbt_t = np.ascontiguousarray(
        pwbt.reshape(4, 128, C).transpose(1, 0, 2).reshape(128, 4 * C))
    pwbb = np.asarray(pwb_b, f32)[tau_rows].reshape(1, C).astype(BF16)

    # --- styleT for sd computation on device: pre-tiled [128, kt, (n, px)] f32 ---
    styleT = np.ascontiguousarray(se.transpose(1, 0, 2, 3).reshape(SD, N * 16)).astype(f32)
    styleT_t = np.ascontiguousarray(
        styleT.reshape(4, 128, N * 16).transpose(1, 0, 2).reshape(128, 4 * N * 16))

    in_maps = []
    for j in range(N):
        pp = np.pad(pred[j], ((0, 0), (1, 1), (1, 1)), mode="reflect").reshape(C, PA)
        sel = np.zeros((128, N), f32)
        sel[:, j] = 1.0
        in_maps.append(
            dict(
                pred_pad=np.ascontiguousarray(pp).astype(BF16),
                patches=patches_t,
                styleT=styleT_t,
                sel=sel,
                dwt=np.ascontiguousarray(dwt_full[:, j * OSH:(j + 1) * OSH]),
                dwb=np.ascontiguousarray(dwb_full[:, j * OSH:(j + 1) * OSH]),
                pwkt=np.ascontiguousarray(pwkt_full[:, j * OSH:(j + 1) * OSH]),
                pwkb=np.ascontiguousarray(pwkb_full[:, j * OSH:(j + 1) * OSH]),
                pwbt=pwbt_t,
                pwbb=pwbb,
            )
        )
    return in_maps


def _unshard(results):
    """results[j]['out'] is [512, 16384] f32 in device channel order."""
    out = np.empty((N, C, H, W), np.float32)
    for j in range(N):
        dev = np.asarray(results[j]["out"], np.float32).reshape(C, H, W)
        for d, t in enumerate(TAU_BLOCK):
            out[j, t * GS:(t + 1) * GS] = dev[d * GS:(d + 1) * GS]
    return out


# how many 512-px N-tiles are accumulated per psum set before evacuation
CONV_NT = 2


def build_nc():
    from concourse import bacc, mybir, tile
    from contextlib import ExitStack

    dt = mybir.dt
    AF = mybir.ActivationFunctionType
    ALU = mybir.AluOpType

    nc = bacc.Bacc(num_devices=N)

    pred_pad = nc.declare_dram_parameter("pred_pad", [C, PA], dt.bfloat16, isOutput=False)
    patches = nc.declare_dram_parameter("patches", [128, 16 * N * NTAP], dt.bfloat16, isOutput=False)
    styleT = nc.declare_dram_parameter("styleT", [128, 4 * N * 16], dt.float32, isOutput=False)
    sel = nc.declare_dram_parameter("sel", [128, N], dt.float32, isOutput=False)
    dwt = nc.declare_dram_parameter("dwt", [KDW, OSH], dt.bfloat16, isOutput=False)
    dwb = nc.declare_dram_parameter("dwb", [1, OSH], dt.bfloat16, isOutput=False)
    pwkt = nc.declare_dram_parameter("pwkt", [SD, OSH], dt.bfloat16, isOutput=False)
    pwkb = nc.declare_dram_parameter("pwkb", [1, OSH], dt.bfloat16, isOutput=False)
    pwbt = nc.declare_dram_parameter("pwbt", [128, 4 * C], dt.bfloat16, isOutput=False)
    pwbb = nc.declare_dram_parameter("pwbb", [1, C], dt.bfloat16, isOutput=False)
    out_dev = nc.declare_dram_parameter("out", [C, HW], dt.float32, isOutput=True)

    replica = [list(range(N))]

    with tile.TileContext(nc) as tc, ExitStack() as ctx:
        dram = ctx.enter_context(tc.tile_pool(name="dram", bufs=1, space="DRAM"))
        gen_all = dram.tile([N, 10, OSH], dt.bfloat16, tag="gall")
        gen_out = dram.tile([N, 10, OSH], dt.bfloat16, tag="gout")

        const_p = ctx.enter_context(tc.tile_pool(name="const", bufs=1))
        pred_p = ctx.enter_context(tc.tile_pool(name="pred", bufs=1))
        stats_p = ctx.enter_context(tc.tile_pool(name="stats", bufs=2))
        fker_p = ctx.enter_context(tc.tile_pool(name="fker", bufs=1))
        fload_p = ctx.enter_context(tc.tile_pool(name="fload", bufs=3))
        stage_p = ctx.enter_context(tc.tile_pool(name="stage", bufs=4))

        # ------------------------------------------------ constant-ish loads
        pt = const_p.tile([128, 16, N * NTAP], dt.bfloat16, tag="pt")
        nc.sync.dma_start(out=pt[:], in_=patches.rearrange("p (kt m) -> p kt m", kt=16))

        st = const_p.tile([128, 4, N * 16], dt.float32, tag="st")
        nc.sync.dma_start(out=st[:], in_=styleT.rearrange("p (kt m) -> p kt m", kt=4))

        sel_sb = const_p.tile([128, N], dt.float32, tag="sel")
        nc.sync.dma_start(out=sel_sb[:], in_=sel[:, :])

        pwbt_sb = const_p.tile([128, 4, C], dt.bfloat16, tag="pwbt")
        nc.sync.dma_start(out=pwbt_sb[:], in_=pwbt.rearrange("p (kt m) -> p kt m", kt=4))

        pwbb_sb = const_p.tile([1, C], dt.bfloat16, tag="pwbb")
        nc.sync.dma_start(out=pwbb_sb[:], in_=pwbb[:, :])

        ones = const_p.tile([1, 128], dt.bfloat16, tag="ones")
        nc.vector.memset(ones[:], 1.0)

        # ------------------------------------------------ predicted (padded, bf16)
        # pair 0 loads first (stats p0 starts early); pairs 1-3 are issued
        # later so the dwt weight stream isn't starved at kernel start.
        img = [pred_p.tile([128, PA], dt.bfloat16, name=f"img{p}", tag=f"img{p}")
               for p in range(4)]
        img_loaded = [False] * 4

        def load_img(p):
            if not img_loaded[p]:
                nc.gpsimd.dma_start(out=img[p][:], in_=pred_pad[p * 128:(p + 1) * 128, :])
                img_loaded[p] = True

        load_img(0)

        # ------------------------------------------------ sd = mean(style, px)
        sdf = const_p.tile([128, 4, N], dt.float32, tag="sdf")
        sdb = const_p.tile([128, 4, N], dt.bfloat16, tag="sdb")
        sdnb = const_p.tile([128, 4], dt.bfloat16, tag="sdnb")  # own-sample column
        tmp8 = stats_p.tile([128, N], dt.float32, tag="tmp8")
        sdn_f = const_p.tile([128, 4], dt.float32, tag="sdnf")
        for kt in range(4):
            nc.vector.tensor_reduce(
                out=sdf[:, kt, :],
                in_=st[:, kt, :].rearrange("p (n x) -> p n x", x=16),
                axis=mybir.AxisListType.X,
                op=ALU.add,
            )
            nc.vector.tensor_scalar(
                out=sdb[:, kt, :], in0=sdf[:, kt, :], scalar1=1.0 / 16.0,
                scalar2=None, op0=ALU.mult,
            )
            # own sample's sd column (via one-hot sel): sdn = sum_n sdf[:,n]*sel[:,n]
            nc.vector.tensor_tensor(
                out=tmp8[:], in0=sdf[:, kt, :], in1=sel_sb[:], op=ALU.mult
            )
            nc.vector.tensor_reduce(
                out=sdn_f[:, kt:kt + 1], in_=tmp8[:], axis=mybir.AxisListType.X, op=ALU.add
            )
        nc.vector.tensor_scalar(
            out=sdnb[:], in0=sdn_f[:], scalar1=1.0 / 16.0, scalar2=None, op0=ALU.mult
        )

        # ------------------------------------------------ generator phase (PE)
        with tc.tile_pool(name="psgen", bufs=8, space="PSUM") as psum_g, \
             tc.tile_pool(name="wstream", bufs=2) as wstream_p, \
             tc.tile_pool(name="gstg", bufs=2) as gstg_p:
            # dw generator
            ps_dw = [psum_g.tile([128, 512], dt.float32, name=f"dwg{b}", tag="g")
                     for b in range(8)]
            for kt in range(16):
                wt = wstream_p.tile([128, OSH], dt.bfloat16, name="wt", tag="w")
                nc.sync.dma_start(out=wt[:], in_=dwt[kt * 128:(kt + 1) * 128, :])
                for b in range(8):
                    nc.tensor.matmul(
                        out=ps_dw[b][:N * NTAP, :],
                        lhsT=pt[:, kt, :],
                        rhs=wt[:, b * 512:(b + 1) * 512],
                        start=(kt == 0), stop=False,
                    )
            for b in range(8):
                bt = gstg_p.tile([1, 512], dt.bfloat16, name="bt", tag="bias")
                nc.sync.dma_start(out=bt[:], in_=dwb[0:1, b * 512:(b + 1) * 512])
                nc.tensor.matmul(
                    out=ps_dw[b][:N * NTAP, :],
                    lhsT=ones[:1, :N * NTAP],
                    rhs=bt[:1, :],
                    start=False, stop=True,
                )
                gsb = gstg_p.tile([N * NTAP, 512], dt.bfloat16, name="gsb", tag="gs")
                nc.scalar.copy(out=gsb[:], in_=ps_dw[b][:N * NTAP, :])
                nc.sync.dma_start(
                    out=gen_all[:, 0:NTAP, b * 512:(b + 1) * 512],
                    in_=gsb[:, :],
                )

            # pwk generator
            ps_pk = [psum_g.tile([128, 512], dt.float32, name=f"pkg{b}", tag="g")
                     for b in range(8)]
            for kt in range(4):
                wt = wstream_p.tile([128, OSH], dt.bfloat16, name="wt", tag="w")
                nc.sync.dma_start(out=wt[:], in_=pwkt[kt * 128:(kt + 1) * 128, :])
                for b in range(8):
                    nc.tensor.matmul(
                        out=ps_pk[b][:N, :],
                        lhsT=sdb[:, kt, :],
                        rhs=wt[:, b * 512:(b + 1) * 512],
                        start=(kt == 0), stop=False,
                    )
            for b in range(8):
                bt = gstg_p.tile([1, 512], dt.bfloat16, name="bt", tag="bias")
                nc.sync.dma_start(out=bt[:], in_=pwkb[0:1, b * 512:(b + 1) * 512])
                nc.tensor.matmul(
                    out=ps_pk[b][:N, :],
                    lhsT=ones[:1, :N],
                    rhs=bt[:1, :],
                    start=False, stop=True,
                )
                g2sb = gstg_p.tile([N, 512], dt.bfloat16, name="g2sb", tag="gs")
                nc.scalar.copy(out=g2sb[:], in_=ps_pk[b][:N, :])
                nc.sync.dma_start(
                    out=gen_all[:, NTAP, b * 512:(b + 1) * 512], in_=g2sb[:]
                )

            # pwb bias chain (device channel order)
            pwb_sb = const_p.tile([128, 4], dt.float32, tag="pwbv")
            for m in range(4):
                pm = psum_g.tile([128, 512], dt.float32, name="pwbps", tag="g")
                for kt in range(4):
                    nc.tensor.matmul(
                        out=pm[:, 0:1],
                        lhsT=pwbt_sb[:, kt, m * 128:(m + 1) * 128],
                        rhs=sdnb[:, kt:kt + 1],
                        start=(kt == 0), stop=False,
                    )
                nc.tensor.matmul(
                    out=pm[:, 0:1],
                    lhsT=pwbb_sb[:1, m * 128:(m + 1) * 128],
                    rhs=ones[:1, 0:1],
                    start=False, stop=True,
                )
                nc.scalar.copy(out=pwb_sb[:, m:m + 1], in_=pm[:, 0:1])

            nc.gpsimd.collective_compute(
                "AllToAll",
                ALU.bypass,
                replica_groups=replica,
                ins=[gen_all[:, :, :].opt()],
                outs=[gen_out[:, :, :].opt()],
            )

        # ------------------------------------------------ instance-norm stats
        # sum-of-squares on VectorE (STT + accum, 8 chunks); sums on ScalarE
        # (Copy activation + accum_out, 8 chunks); var = E[x^2] - mu^2.
        rstd_sb = const_p.tile([128, 4], dt.float32, tag="rstd")
        muneg_sb = const_p.tile([128, 4], dt.bfloat16, tag="muneg")

        def stats_pair(p):
            view = img[p].rearrange("p (r c) -> p r c", c=PW)
            acc8 = stats_p.tile([128, 8], dt.float32, name="acc8", tag="acc8")
            sqd = stats_p.tile([128, 16, 128], dt.float32, name="sqd", tag="sqd", bufs=1)
            for j in range(8):
                xs = view[:, 1 + 16 * j:1 + 16 * (j + 1), 1:129]
                nc.vector.scalar_tensor_tensor(
                    out=sqd[:], in0=xs, scalar=1.0, in1=xs,
                    op0=ALU.mult, op1=ALU.mult,
                    accum_out=acc8[:, j:j + 1],
                )
            ssum = stats_p.tile([128, 1], dt.float32, name="ssum", tag="ssum")
            nc.vector.tensor_reduce(
                out=ssum[:], in_=view[:, 1:129, 1:129],
                axis=mybir.AxisListType.XY, op=ALU.add
            )
            ssq = stats_p.tile([128, 1], dt.float32, name="ssq", tag="ssq")
            nc.vector.tensor_reduce(
                out=ssq[:], in_=acc8[:], axis=mybir.AxisListType.X, op=ALU.add
            )
            mu = stats_p.tile([128, 1], dt.float32, name="mu", tag="mu")
            nc.vector.tensor_scalar(
                out=mu[:], in0=ssum[:], scalar1=1.0 / HW, scalar2=None, op0=ALU.mult
            )
            nc.vector.tensor_scalar(
                out=muneg_sb[:, p:p + 1], in0=mu[:], scalar1=-1.0,
                scalar2=None, op0=ALU.mult,
            )
            ex2 = stats_p.tile([128, 1], dt.float32, name="ex2", tag="ex2")
            nc.vector.tensor_scalar(
                out=ex2[:], in0=ssq[:], scalar1=1.0 / HW, scalar2=None, op0=ALU.mult
            )
            mu2 = stats_p.tile([128, 1], dt.float32, name="mu2", tag="mu2")
            nc.vector.tensor_tensor(out=mu2[:], in0=mu[:], in1=mu[:], op=ALU.mult)
            varp = stats_p.tile([128, 1], dt.float32, name="varp", tag="varp")
            nc.vector.tensor_tensor(out=varp[:], in0=ex2[:], in1=mu2[:], op=ALU.subtract)
            vtmp = stats_p.tile([128, 1], dt.float32, name="vtmp", tag="sm2")
            nc.vector.tensor_scalar(
                out=vtmp[:], in0=varp[:], scalar1=VAR_CORR, scalar2=EPS,
                op0=ALU.mult, op1=ALU.add,
            )
            stdt = stats_p.tile([128, 1], dt.float32, name="stdt", tag="sm3")
            nc.scalar.sqrt(stdt[:], vtmp[:])
            nc.vector.reciprocal(out=rstd_sb[:, p:p + 1], in_=stdt[:])

        # ------------------------------------------------ per-half: F build + conv
        with tc.tile_pool(name="pssml", bufs=2, space="PSUM") as psum_s, \
             tc.tile_pool(name="psconv", bufs=6, space="PSUM") as psum_c:
            for h in range(2):
                pA, pB = 2 * h, 2 * h + 1
                load_img(pA)
                load_img(pB)
                stats_pair(pA)
                stats_pair(pB)

                # fused, scaled kernels F~ : [part(ci), tap, geo, co2] bf16
                fsb = fker_p.tile([128, NTAP, 2, GS], dt.bfloat16,
                                  name=f"fsb{h}", tag=f"fsb{h}")
                for t in range(NTAP):
                    w128 = fload_p.tile([128, 2, GS], dt.bfloat16, name="wld", tag="wld")
                    p128 = fload_p.tile([128, 2, GS], dt.bfloat16, name="pld", tag="pld")
                    nc.sync.dma_start(
                        out=w128[0:64, :, :],
                        in_=gen_out[4 * h:4 * h + 3:2, t, :].rearrange(
                            "g (cm ci) -> cm g ci", ci=GS),
                    )
                    nc.sync.dma_start(
                        out=w128[64:128, :, :],
                        in_=gen_out[4 * h + 1:4 * h + 4:2, t, :].rearrange(
                            "g (cm ci) -> cm g ci", ci=GS),
                    )
                    nc.sync.dma_start(
                        out=p128[0:64, :, :],
                        in_=gen_out[4 * h:4 * h + 3:2, NTAP, :].rearrange(
                            "g (cm co) -> cm g co", co=GS),
                    )
                    nc.sync.dma_start(
                        out=p128[64:128, :, :],
                        in_=gen_out[4 * h + 1:4 * h + 4:2, NTAP, :].rearrange(
                            "g (cm co) -> cm g co", co=GS),
                    )
                    psA = psum_s.tile([128, 512], dt.float32, name="fbA", tag="s")
                    psB = psum_s.tile([128, 512], dt.float32, name="fbB", tag="s")
                    # F^T = W^T @ P^T  (fp32), per group
                    nc.tensor.matmul(out=psA[0:64, :GS], lhsT=w128[0:64, 0, :],
                                     rhs=p128[0:64, 0, :], start=True, stop=True)
                    nc.tensor.matmul(out=psA[64:128, :GS], lhsT=w128[64:128, 0, :],
                                     rhs=p128[64:128, 0, :], start=True, stop=True)
                    nc.tensor.matmul(out=psB[0:64, :GS], lhsT=w128[0:64, 1, :],
                                     rhs=p128[0:64, 1, :], start=True, stop=True)
                    nc.tensor.matmul(out=psB[64:128, :GS], lhsT=w128[64:128, 1, :],
                                     rhs=p128[64:128, 1, :], start=True, stop=True)
                    # scale by rstd (per input channel) + cast bf16
                    nc.vector.tensor_scalar(out=fsb[0:64, t, 0, :], in0=psA[0:64, :GS],
                                            scalar1=rstd_sb[0:64, pA:pA + 1],
                                            scalar2=None, op0=ALU.mult)
                    nc.vector.tensor_scalar(out=fsb[64:128, t, 0, :], in0=psA[64:128, :GS],
                                            scalar1=rstd_sb[64:128, pA:pA + 1],
                                            scalar2=None, op0=ALU.mult)
                    nc.vector.tensor_scalar(out=fsb[0:64, t, 1, :], in0=psB[0:64, :GS],
                                            scalar1=rstd_sb[0:64, pB:pB + 1],
                                            scalar2=None, op0=ALU.mult)
                    nc.vector.tensor_scalar(out=fsb[64:128, t, 1, :], in0=psB[64:128, :GS],
                                            scalar1=rstd_sb[64:128, pB:pB + 1],
                                            scalar2=None, op0=ALU.mult)

                # B bias accumulation (psum holds -B); pair B quadrant-swapped
                bpsA = psum_s.tile([128, 512], dt.float32, name="bpsA", tag="s")
                bpsB = psum_s.tile([128, 512], dt.float32, name="bpsB", tag="s")
                for t in range(NTAP):
                    st_ = (t == 0)
                    sp_ = (t == NTAP - 1)
                    nc.tensor.matmul(out=bpsA[0:64, 0:1], lhsT=fsb[0:64, t, 0, :],
                                     rhs=muneg_sb[0:64, pA:pA + 1], start=st_, stop=sp_,
                                     skip_group_check=True)
                    nc.tensor.matmul(out=bpsA[64:128, 0:1], lhsT=fsb[64:128, t, 0, :],
                                     rhs=muneg_sb[64:128, pA:pA + 1], start=st_, stop=sp_,
                                     skip_group_check=True)
                    nc.tensor.matmul(out=bpsB[64:128, 0:1], lhsT=fsb[0:64, t, 1, :],
                                     rhs=muneg_sb[0:64, pB:pB + 1], start=st_, stop=sp_,
                                     skip_group_check=True)
                    nc.tensor.matmul(out=bpsB[0:64, 0:1], lhsT=fsb[64:128, t, 1, :],
                                     rhs=muneg_sb[64:128, pB:pB + 1], start=st_, stop=sp_,
                                     skip_group_check=True)
                bias_A = const_p.tile([128, 1], dt.float32, name=f"biasA{h}", tag=f"bA{h}")
                bias_B = const_p.tile([128, 1], dt.float32, name=f"biasB{h}", tag=f"bB{h}")
                nc.vector.tensor_tensor(out=bias_A[:], in0=bpsA[:, 0:1],
                                        in1=pwb_sb[:, pA:pA + 1], op=ALU.add)
                nc.vector.tensor_tensor(out=bias_B[:], in0=bpsB[:, 0:1],
                                        in1=pwb_sb[:, pB:pB + 1], op=ALU.add)

                if h == 0:
                    load_img(2)
                    load_img(3)

                # ------------------------------------------------ the conv
                imA = img[pA].rearrange("p (r c) -> p r c", c=PW)
                imB = img[pB].rearrange("p (r c) -> p r c", c=PW)
                NBLK = HW // (512 * CONV_NT)
                for blk in range(NBLK):
                    y0 = blk * 4 * CONV_NT
                    psA_ = [psum_c.tile([128, 512], dt.float32, name="cvA", tag="c")
                            for _ in range(CONV_NT)]
                    psB_ = [psum_c.tile([128, 512], dt.float32, name="cvB", tag="c")
                            for _ in range(CONV_NT)]
                    for t in range(NTAP):
                        ky, kx = t // 3, t % 3
                        st_ = (t == 0)
                        sp_ = (t == NTAP - 1)
                        for nt in range(CONV_NT):
                            y = y0 + 4 * nt
                            rA = imA[:, y + ky:y + ky + 4, kx:kx + 128]
                            rB = imB[:, y + ky:y + ky + 4, kx:kx + 128]
                            nc.tensor.matmul(out=psA_[nt][0:64, :], lhsT=fsb[0:64, t, 0, :],
                                             rhs=rA[0:64], start=st_, stop=sp_,
                                             skip_group_check=True)
                            nc.tensor.matmul(out=psA_[nt][64:128, :], lhsT=fsb[64:128, t, 0, :],
                                             rhs=rA[64:128], start=st_, stop=sp_,
                                             skip_group_check=True)
                            nc.tensor.matmul(out=psB_[nt][64:128, :], lhsT=fsb[0:64, t, 1, :],
                                             rhs=rB[0:64], start=st_, stop=sp_,
                                             skip_group_check=True)
                            nc.tensor.matmul(out=psB_[nt][0:64, :], lhsT=fsb[64:128, t, 1, :],
                                             rhs=rB[64:128], start=st_, stop=sp_,
                                             skip_group_check=True)
                    # evacuate + bias; ScalarE takes pair A, VectorE pair B
                    stA = stage_p.tile([128, 512 * CONV_NT], dt.float32, name="stA", tag="stg")
                    stB = stage_p.tile([128, 512 * CONV_NT], dt.float32, name="stB", tag="stg")
                    for nt in range(CONV_NT):
                        nc.scalar.activation(
                            out=stA[:, nt * 512:(nt + 1) * 512], in_=psA_[nt][:],
                            func=AF.Identity, bias=bias_A[:, 0:1], scale=1.0,
                        )
                        nc.scalar.activation(
                            out=stB[:, nt * 512:(nt + 1) * 512], in_=psB_[nt][:],
                            func=AF.Identity, bias=bias_B[:, 0:1], scale=1.0,
                        )
                    px0 = y0 * 128
                    nc.sync.dma_start(
                        out=out_dev[pA * 128:(pA + 1) * 128, px0:px0 + 512 * CONV_NT],
                        in_=stA[:],
                    )
                    nc.sync.dma_start(
                        out=out_dev[pB * 128:(pB + 1) * 128, px0:px0 + 512 * CONV_NT],
                        in_=stB[:],
                    )

    nc.compile()
    return nc


_NC_CACHE = {}


def kernel(**inputs) -> np.ndarray:
    from concourse.bass_utils import run_bass_kernel_spmd

    in_maps = _host_prep(**inputs)
    if "nc" not in _NC_CACHE:
        _NC_CACHE["nc"] = build_nc()
    nc = _NC_CACHE["nc"]
    res = run_bass_kernel_spmd(nc, in_maps, core_ids=list(range(N)))
    return _unshard(res.results)


if __name__ == "__main__":
    import jax

    import reference

    with jax.default_device(jax.devices("cpu")[0]):
        inputs = {k: np.asarray(v) for k, v in reference.setup_inputs().items()}
        expected = np.asarray(reference.reference(**inputs))
    actual = kernel(**inputs)
    err = np.sqrt(((actual - expected) ** 2).mean()) / np.sqrt((expected ** 2).mean())
    print("Relative error:", err)
